# revision 1
# baseline (speedup 1.0000x reference)
"""Trainium2 Bass kernel for nn_ADSREncoderV3 (8-core data-parallel).

Pipeline (per core, 4 of 32 samples):
  1. log-RMS envelope of wav (DMA-bound, fused square+reduce on DVE)
  2. conv1(k=3) + GELU as a single K=12 block-diagonal matmul
  3. 4x dsconv blocks: depthwise+pointwise fused into 3 shifted matmuls
     (PSUM-accumulated), training-mode BatchNorm with cross-core stats
     (AllGather of per-core partial sums), SiLU via one ACT op with the
     BN affine folded into scale/bias.
  4. note gather + length-weighted pooling as PSUM-accumulated matmuls
     with wnt-scaled identity weights (note layout baked at build time
     from the actual onset_flags, which are identical across samples).

Output: (32, 64, 512) f32; columns >= max note length are zero.
"""

import os
import numpy as np

HOP = 512
N_MAX = 16
EPS = 1e-7
BN_EPS = 1e-5
B = 32
T_WAV = 262144
P = 512
N_CORES = 8
B_LOC = B // N_CORES  # 4
PAD = 16  # zero pad columns on each side of x tiles (max dilation is 8)

_CACHE = {}
_MAGIC = 0x5F3759DF  # rsqrt seed


# ---------------------------------------------------------------- host plan


def _note_plan(flags_row):
    """Replicates reference note bookkeeping for one onset pattern.

    Returns (offsets, lengths, wnt) for the first N_MAX notes."""
    pos = np.nonzero(flags_row)[0]
    if len(pos) == 0:
        return [], [], []
    pos = pos.tolist()
    ends = pos[1:] + [P]
    offs, lens = [], []
    for n, (o, e) in enumerate(zip(pos, ends)):
        if n >= N_MAX:
            break
        offs.append(int(o))
        lens.append(int(e - o))
    tot = float(sum(lens)) + EPS
    wnt = [l / tot for l in lens]
    return offs, lens, wnt


def _numpy_reference(wav, onset_flags, w0, b0, dws, pws, gs, bts):
    """Exact numpy fallback (used only if inputs deviate from the
    expected uniform-onset-pattern shape)."""
    Bn, _, Tn = wav.shape
    Pn = Tn // HOP
    rms = np.sqrt(np.mean((wav * wav).reshape(Bn, 1, Pn, HOP), axis=-1) + EPS)
    x = np.log(rms + EPS)  # (B,1,P)

    def conv1d(x, w, b=None, dilation=1, groups=1):
        k = w.shape[-1]
        pad = (k - 1) // 2 * dilation
        Bi, Ci, Pi = x.shape
        Co = w.shape[0]
        xp = np.pad(x, ((0, 0), (0, 0), (pad, pad)))
        y = np.zeros((Bi, Co, Pi), np.float32)
        cig = Ci // groups
        cog = Co // groups
        for g in range(groups):
            xs = xp[:, g * cig:(g + 1) * cig]
            wg = w[g * cog:(g + 1) * cog]
            for kk in range(k):
                seg = xs[:, :, kk * dilation: kk * dilation + Pi]
                y[:, g * cog:(g + 1) * cog] += np.einsum(
                    "bip,oi->bop", seg, wg[:, :, kk])
        if b is not None:
            y += b[None, :, None]
        return y

    from math import erf
    verf = np.vectorize(lambda v: erf(v), otypes=[np.float64])
    y = conv1d(x, w0, b0)
    x = (0.5 * y * (1.0 + verf(y / np.sqrt(2.0)))).astype(np.float32)
    for dw, pw, g, bt in zip(dws, pws, gs, bts):
        c = x.shape[1]
        d = {0: 1, 1: 2, 2: 4, 3: 8}[dws.index(dw)]
        x = conv1d(x, dw, dilation=d, groups=c)
        x = conv1d(x, pw)
        mu = x.mean(axis=(0, 2), keepdims=True)
        var = x.var(axis=(0, 2), keepdims=True)
        x = (x - mu) / np.sqrt(var + BN_EPS) * g[None, :, None] + bt[None, :, None]
        x = x / (1.0 + np.exp(-x)) * 1.0 + 0.0 if False else x * (1.0 / (1.0 + np.exp(-x)))
    # gather
    out = np.zeros((Bn, x.shape[1], Pn), np.float32)
    for b_i in range(Bn):
        offs, lens, wnt = _note_plan(onset_flags[b_i, 0])
        for o, l, w in zip(offs, lens, wnt):
            out[b_i, :, :l] += w * x[b_i, :, o:o + l]
    return out


def _pack_consts(w0, b0, dws, pws, gs, bts, wnt):
    """Build all device constant tensors (f32)."""
    f32 = np.float32
    # conv1: lhsT (12,128): row q=(t,tau) -> col m=(t,co); 0.5 folds log(sqrt)
    w0h = 0.5 * w0[:, 0, :]  # (32, 3)
    w0lT = np.zeros((12, 128), f32)
    for t in range(4):
        for tau in range(3):
            w0lT[3 * t + tau, 32 * t:32 * t + 32] = w0h[:, tau]
    b0c = np.tile(b0.astype(f32), 4).reshape(128, 1)

    # layer 1: (3, 64, 128): row (t_loc, ci32) -> col (t_loc, co64)
    M1 = [pws[0][:, :, 0] * dws[0][None, :, 0, k] for k in range(3)]  # (64,32)
    l1T = np.zeros((3, 64, 128), f32)
    for k in range(3):
        for t in range(2):
            l1T[k, 32 * t:32 * t + 32, 64 * t:64 * t + 64] = M1[k].T

    lnT = []
    for n in (1, 2, 3):
        Mk = [pws[n][:, :, 0] * dws[n][None, :, 0, k] for k in range(3)]  # (64,64)
        lt = np.zeros((3, 128, 128), f32)
        for k in range(3):
            for t in range(2):
                lt[k, 64 * t:64 * t + 64, 64 * t:64 * t + 64] = Mk[k].T
        lnT.append(lt)

    bnp = np.zeros((4, 128, 2), f32)
    for n in range(4):
        bnp[n, :, 0] = np.tile(gs[n], 2)
        bnp[n, :, 1] = np.tile(bts[n], 2)

    NN = len(wnt)
    gidT = np.zeros((max(NN, 1), 128, 128), f32)
    for j in range(NN):
        np.fill_diagonal(gidT[j], wnt[j])
    # fold matrix: out[(u',c')] = sum_u in[(u,c')]
    foldT = np.tile(np.eye(64, dtype=f32), (2, 2))
    return dict(w0lT=w0lT, b0c=b0c, l1T=l1T, l2T=lnT[0], l3T=lnT[1],
                l4T=lnT[2], bnp=bnp, gidT=gidT, foldT=foldT)


# ---------------------------------------------------------------- device


def _build(plan_key, loop=1, no_cc=False, phase='full'):
    """Build the SPMD Bass program for a given note plan."""
    import concourse.bacc as bacc
    import concourse.mybir as mybir
    import concourse.tile as tile
    from concourse.bass import ts, ds, _add_dep_helper  # noqa: F401

    offs, lens, srt = plan_key[0], plan_key[1], None
    notes = sorted(zip(plan_key[0], plan_key[1], range(len(plan_key[0]))),
                   key=lambda x: -x[1])  # by length desc
    ML = notes[0][1]
    NN = len(notes)

    f32 = mybir.dt.float32
    i32 = mybir.dt.int32
    AF = mybir.ActivationFunctionType
    ALU = mybir.AluOpType
    AX = mybir.AxisListType

    nc = bacc.Bacc("TRN2", target_bir_lowering=False, debug=False,
                   num_devices=N_CORES, num_swdge_queues=4)

    wav = nc.declare_dram_parameter("wav", [B_LOC, T_WAV], f32, isOutput=False)
    p_w0lT = nc.declare_dram_parameter("w0lT", [12, 128], f32, isOutput=False)
    p_b0c = nc.declare_dram_parameter("b0c", [128, 1], f32, isOutput=False)
    p_l1T = nc.declare_dram_parameter("l1T", [3, 64, 128], f32, isOutput=False)
    p_l2T = nc.declare_dram_parameter("l2T", [3, 128, 128], f32, isOutput=False)
    p_l3T = nc.declare_dram_parameter("l3T", [3, 128, 128], f32, isOutput=False)
    p_l4T = nc.declare_dram_parameter("l4T", [3, 128, 128], f32, isOutput=False)
    p_bnp = nc.declare_dram_parameter("bnp", [4, 128, 2], f32, isOutput=False)
    p_gidT = nc.declare_dram_parameter("gidT", [max(NN, 1), 128, 128], f32,
                                       isOutput=False)
    p_foldT = nc.declare_dram_parameter("foldT", [128, 128], f32,
                                        isOutput=False)
    out_ext = nc.declare_dram_parameter("out", [B_LOC, 64, ML], f32,
                                        isOutput=True)
    dbg_ext = (nc.declare_dram_parameter("dbg", [128, 18], f32, isOutput=True)
               if os.environ.get("KERNEL_DBG") == "1" else None)

    with tile.TileContext(nc) as tc:
        with (
            tc.tile_pool(name="cpool", bufs=1) as cpool,
            tc.tile_pool(name="wpool", bufs=2) as wpool,
            tc.tile_pool(name="spool", bufs=2) as spool,
            tc.tile_pool(name="epool", bufs=1) as epool,
            tc.tile_pool(name="xpool", bufs=2) as xpool,
            tc.tile_pool(name="tpool", bufs=2) as tpool,
            tc.tile_pool(name="pspool", bufs=4, space="PSUM") as pspool,
            tc.tile_pool(name="popool", bufs=2, space="PSUM") as popool,
            tc.tile_pool(name="dpool", bufs=1, space="DRAM") as dpool,
        ):
            # ---- constants (scalar-engine HWDGE ring, issued first) ----
            c_w0lT = cpool.tile([12, 128], f32, name="c_w0lT")
            nc.scalar.dma_start(out=c_w0lT[:, :], in_=p_w0lT[:, :])
            c_b0c = cpool.tile([128, 1], f32, name="c_b0c")
            nc.scalar.dma_start(out=c_b0c[:, :], in_=p_b0c[:, :])
            c_l1 = []
            for k in range(3):
                t_ = cpool.tile([64, 128], f32, name=f"c_l1_{k}")
                nc.scalar.dma_start(out=t_[:, :], in_=p_l1T[k])
                c_l1.append(t_)
            c_ln = {}
            for n, p_l in ((2, p_l2T), (3, p_l3T), (4, p_l4T)):
                for k in range(3):
                    t_ = cpool.tile([128, 128], f32, name=f"c_l{n}_{k}")
                    nc.scalar.dma_start(out=t_[:, :], in_=p_l[k])
                    c_ln[(n, k)] = t_
            c_bnp = []
            for n in range(4):
                t_ = cpool.tile([128, 2], f32, name=f"c_bnp{n}")
                nc.scalar.dma_start(out=t_[:, :], in_=p_bnp[n])
                c_bnp.append(t_)
            c_fold = cpool.tile([128, 128], f32, name="c_fold")
            nc.scalar.dma_start(out=c_fold[:, :], in_=p_foldT[:, :])
            c_gid = []
            for j in range(NN):
                t_ = cpool.tile([128, 128], f32, name=f"c_gid{j}")
                nc.scalar.dma_start(out=t_[:, :], in_=p_gidT[j])
                c_gid.append(t_)

            # ---- ACT table preload (natural_log set) ----
            dmy = epool.tile([1, 1], f32, name="dmy")
            dmyo = epool.tile([1, 1], f32, name="dmyo")
            nc.vector.memset(dmy[:, :], 1.0)
            nc.scalar.activation(dmyo[:, :], dmy[:, :], AF.Ln)
            c_eps = epool.tile([128, 1], f32, name="c_eps")
            nc.vector.memset(c_eps[:, :], EPS)
            recv_sem = nc.alloc_semaphore("xch_recv")
            send_sem = nc.alloc_semaphore("xch_send")
            xch_count = [0]
            dbgx = epool.tile([128, 16], f32, name="dbgx")

            def one_pass():
              # ---- envelope ----
              sums = epool.tile([128, 16], f32, name="sums")
              use_rdma = (os.environ.get("KERNEL_EXCHANGE", "cc") == "rdma"
                          and not no_cc and phase in ('full', 'layers'))
              part2s, xchs = {}, {}
              if use_rdma:
                  slot_off = nc.gpsimd.partition_id() * 2
                  for ne in range(1, 5):
                      part2s[ne] = tpool.tile([128, 2], f32,
                                              name=f"part2_{ne}",
                                              tag=f"part2_{ne}")
                      xchs[ne] = tpool.tile([128, 16], f32,
                                            name=f"xch{ne}",
                                            tag=f"xch{ne}")
                  for ne in range(1, 5):
                      nc.gpsimd.remote_dma_broadcast(
                          out_ap=xchs[ne][:, ds(slot_off, 2)],
                          in_ap=part2s[ne][:, :],
                          remote_sem=recv_sem, local_sem=send_sem,
                          rdests=[(0, k) for k in range(8)],
                          queue_num=ne - 1)
              if phase == 'nop':
                  return
              for t in range(B_LOC):
                  wt = wpool.tile([128, 2048], f32, name="wt")
                  nc.sync.dma_start(
                      out=wt[:, :],
                      in_=wav[t].rearrange("(p f) -> p f", p=128))
                  for k in range(4):
                      sq = spool.tile([128, 512], f32, name="sq")
                      nc.scalar.activation(
                          sq[:, :], wt[:, 512 * k:512 * (k + 1)], AF.Square,
                          accum_out=sums[:, 4 * t + k:4 * t + k + 1])
              if phase == 'env':
                  return
              if phase.startswith('cc'):
                  n_cc = int(phase[2:])
                  kind = os.environ.get('KERNEL_CC_KIND', 'AG')
                  eng = getattr(nc, os.environ.get('KERNEL_CC_ENGINE',
                                                   'gpsimd'))
                  prev = sums
                  for i in range(n_cc):
                      cin = dpool.tile([128, 2], f32, name=f"xcin{i}",
                                       space="DRAM")
                      nc.sync.dma_start(out=cin[:, :], in_=prev[:, 0:2])
                      if kind == 'AG':
                          cout = dpool.tile([1024, 2], f32, name=f"xcout{i}",
                                            space="DRAM", addr_space="Shared")
                          eng.collective_compute(
                              "AllGather", ALU.bypass,
                              replica_groups=[list(range(N_CORES))],
                              ins=[cin[:, :].opt()], outs=[cout[:, :].opt()])
                          back = cout[0:128, :]
                      else:
                          cout = dpool.tile([128, 2], f32, name=f"xcout{i}",
                                            space="DRAM", addr_space="Shared")
                          eng.collective_compute(
                              "AllReduce", ALU.add,
                              replica_groups=[list(range(N_CORES))],
                              ins=[cin[:, :].opt()], outs=[cout[:, :].opt()])
                          back = cout[:, :]
                      nxt = tpool.tile([128, 2], f32, name="xgat", tag="xgat")
                      nc.sync.dma_start(out=nxt[:, :], in_=back)
                      prev = nxt
                  return
              # log_rms = 0.5*ln(sum/512 + eps); the 0.5 is folded into w0.
              lr = epool.tile([128, 16], f32, name="lr")
              nc.scalar.activation(lr[:, :], sums[:, :], AF.Ln,
                                   bias=c_eps[:, :], scale=1.0 / HOP)
              # preload gelu table while lr round-trips through DRAM
              nc.scalar.activation(dmyo[:, :], lr[0:1, 0:1], AF.Gelu)

              lr_dram = dpool.tile([4, 512], f32, name="lr_dram", space="DRAM")
              nc.sync.dma_start(
                  out=lr_dram[:, :].rearrange("t (p k) -> p t k", p=128),
                  in_=lr[:, :].rearrange("p (t k) -> p t k", k=4))

              # ---- conv1 + gelu ----
              rhs1 = epool.tile([12, 544], f32, name="rhs1")
              nc.vector.memset(rhs1[:, :], 0.0)
              for tau in range(3):
                  src_lo = max(0, tau - 1)
                  src_hi = min(512, 511 + tau)
                  dst_lo = PAD + src_lo - (tau - 1)
                  L = src_hi - src_lo
                  nc.sync.dma_start(
                      out=rhs1[:, dst_lo:dst_lo + L].rearrange(
                          "(t u) f -> u t f", u=3)[tau],
                      in_=lr_dram[:, src_lo:src_hi])
              ps1 = pspool.tile([128, 512], f32, name="ps", tag="ps")
              nc.tensor.matmul(ps1[:, :], c_w0lT[:, :], rhs1[:, PAD:PAD + 512],
                               start=True, stop=True)
              x1a = xpool.tile([64, 544], f32, name="x1a", bufs=1)
              x1b = xpool.tile([64, 544], f32, name="x1b", bufs=1)
              for xh, lohi in ((x1a, (0, 64)), (x1b, (64, 128))):
                  nc.vector.memset(xh[:, 0:PAD], 0.0)
                  nc.vector.memset(xh[:, PAD + 512:], 0.0)
                  nc.scalar.activation(xh[:, PAD:PAD + 512],
                                       ps1[lohi[0]:lohi[1], :], AF.Gelu,
                                       bias=c_b0c[lohi[0]:lohi[1], :], scale=1.0)
              # preload silu table (hidden under layer-1 matmuls + stats AG)
              nc.scalar.activation(dmyo[:, :], x1a[0:1, PAD:PAD + 1], AF.Silu)

              if phase == 'conv1':
                  return
              # ---- dsconv layers ----
              xa, xb = x1a, x1b
              for n in range(1, 5):
                  d = 1 << (n - 1)
                  lhs = c_l1 if n == 1 else [c_ln[(n, k)] for k in range(3)]
                  stat = tpool.tile([128, 4], f32, name="stat", tag="stat")
                  pss = []
                  for Ti in range(2):
                      ps = pspool.tile([128, 512], f32, name="ps", tag="ps")
                      pss.append(ps)
                      if n == 1:
                          rr = (x1a, x1b)[Ti]
                          rhs = [rr[:, PAD + o:PAD + o + 512] for o in (-d, 0, d)]
                      else:
                          rr = (xa, xb)[Ti]
                          rhs = [rr[:, PAD + o:PAD + o + 512] for o in (-d, 0, d)]
                      nc.tensor.matmul(ps[:, :], lhs[1][:, :], rhs[1],
                                       start=True, stop=False)
                      nc.tensor.matmul(ps[:, :], lhs[0][:, :], rhs[0],
                                       start=False, stop=False)
                      nc.tensor.matmul(ps[:, :], lhs[2][:, :], rhs[2],
                                       start=False, stop=True)
                      nc.vector.reduce_sum(out=stat[:, Ti:Ti + 1], in_=ps[:, :],
                                           axis=AX.X)
                      sq = spool.tile([128, 512], f32, name="sq", tag="sq")
                      nc.scalar.activation(sq[:, :], ps[:, :], AF.Square,
                                           accum_out=stat[:, 2 + Ti:3 + Ti])
                  # fold [sumA,sumB,sqA,sqB] -> (sum,sq) over tiles, then halves
                  part = tpool.tile([128, 2], f32, name="part", tag="part")
                  sview = stat[:, :].rearrange("p (a b) -> p a b", a=2)
                  nc.vector.tensor_tensor(out=part[:, :], in0=sview[:, :, 0],
                                          in1=sview[:, :, 1], op=ALU.add)
                  pf = pspool.tile([128, 2], f32, name="pf", tag="pf", bufs=2)
                  nc.tensor.matmul(pf[:, :], c_fold[:, :], part[:, :],
                                   start=True, stop=True)
                  part2 = (part2s[n] if use_rdma else
                           tpool.tile([128, 2], f32, name="part2",
                                      tag="part2"))
                  p2prod = nc.scalar.copy(out=part2[:, :], in_=pf[:, :])
                  use_cc = os.environ.get("KERNEL_EXCHANGE", "cc") == "cc"
                  glob = tpool.tile([128, 2], f32, name="glob", tag="glob")
                  if no_cc:
                      xch = tpool.tile([128, 16], f32, name="xch", tag="xch")
                      nc.vector.memset(xch[:, :], 0.0)
                      nc.vector.tensor_copy(xch[:, 0:2], part2[:, :])
                      nc.vector.reduce_sum(
                          out=glob[:, :],
                          in_=xch[:, :].rearrange("c (q s) -> c s q", s=2),
                          axis=AX.X)
                  elif use_cc:
                      ccin = dpool.tile([128, 2], f32, name=f"ccin{n}",
                                        space="DRAM")
                      ccout = dpool.tile([1024, 2], f32, name=f"ccout{n}",
                                         space="DRAM", addr_space="Shared")
                      nc.sync.dma_start(out=ccin[:, :], in_=part[:, :])
                      nc.gpsimd.collective_compute(
                          "AllGather", ALU.bypass,
                          replica_groups=[list(range(N_CORES))],
                          ins=[ccin[:, :].opt()], outs=[ccout[:, :].opt()])
                      gath = tpool.tile([128, 32], f32, name="gath",
                                        tag="gath")
                      for half in range(2):
                          nc.sync.dma_start(
                              out=gath[64 * half:64 * half + 64, :].rearrange(
                                  "c (r u s) -> c r u s", u=2, s=2),
                              in_=ccout[:, :].rearrange("(r u c) s -> c r u s",
                                                        u=2, c=64))
                      nc.vector.reduce_sum(
                          out=glob[:, :],
                          in_=gath[:, :].rearrange("c (q s) -> c s q", s=2),
                          axis=AX.X)
                  else:
                      # mesh exchange via remote SBUF-to-SBUF DMA: each core
                      # pushes its partial into slot j of peer (me XOR j).
                      # Cross-die slot attribution lands permuted, which the
                      # sum over slots is invariant to. Descriptors were
                      # pre-generated at pass start; fire the next 7.
                      xch = xchs[n]
                      trig = nc.gpsimd.trigger_dma(count=None,
                                                   queue_num=n - 1)
                      _add_dep_helper(trig.ins, p2prod.ins, sync=True,
                                      reason="rdma reads part2 at trigger")
                      xch_count[0] += 1
                      with tc.tile_critical():
                          nc.vector.wait_ge(recv_sem, 16 * xch_count[0])
                          nc.vector.reduce_sum(
                              out=glob[:, :],
                              in_=xch[:, :].rearrange("c (q s) -> c s q", s=2),
                              axis=AX.X)
                  if dbg_ext is not None and n == 1:
                      nc.sync.dma_start(out=dbg_ext[:, 0:2], in_=part2[:, :])
                      with tc.tile_critical():
                          nc.vector.wait_ge(recv_sem, 14 * xch_count[0])
                          nc.vector.tensor_copy(dbgx[:, :], xch[:, :])
                      nc.sync.dma_start(out=dbg_ext[:, 2:18], in_=dbgx[:, :])
                  # BN affine: A = g*rsqrt(var+eps), Bb = bt - mu*A
                  mom = tpool.tile([128, 2], f32, name="mom", tag="mom")
                  nc.vector.tensor_scalar_mul(mom[:, :], glob[:, :],
                                              1.0 / (B * P))
                  var = tpool.tile([128, 1], f32, name="var", tag="var")
                  nc.vector.tensor_tensor(out=var[:, :], in0=mom[:, 0:1],
                                          in1=mom[:, 0:1], op=ALU.mult)
                  nc.vector.tensor_tensor(out=var[:, :], in0=mom[:, 1:2],
                                          in1=var[:, :], op=ALU.subtract)
                  veps = tpool.tile([128, 1], f32, name="veps", tag="veps")
                  nc.vector.tensor_scalar_add(veps[:, :], var[:, :], BN_EPS)
                  # rsqrt via bit trick + 3 Newton iterations (all DVE)
                  z = tpool.tile([128, 1], f32, name="z", tag="z")
                  h = tpool.tile([128, 1], f32, name="h", tag="h")
                  nc.vector.tensor_scalar(
                      out=h[:, :].bitcast(i32),
                      in0=veps[:, :].bitcast(i32),
                      scalar1=1, scalar2=None,
                      op0=ALU.logical_shift_right)
                  nc.vector.tensor_scalar(
                      out=z[:, :].bitcast(i32),
                      in0=h[:, :].bitcast(i32),
                      scalar1=-1, scalar2=_MAGIC,
                      op0=ALU.mult, op1=ALU.add)
                  for _ in range(3):
                      nc.vector.tensor_tensor(out=h[:, :], in0=z[:, :],
                                              in1=z[:, :], op=ALU.mult)
                      nc.vector.tensor_tensor(out=h[:, :], in0=h[:, :],
                                              in1=veps[:, :], op=ALU.mult)
                      nc.vector.tensor_scalar(out=h[:, :], in0=h[:, :],
                                              scalar1=-0.5, scalar2=1.5,
                                              op0=ALU.mult, op1=ALU.add)
                      nc.vector.tensor_tensor(out=z[:, :], in0=z[:, :],
                                              in1=h[:, :], op=ALU.mult)
                  Acol = tpool.tile([128, 1], f32, name="Acol", tag="Acol")
                  Bcol = tpool.tile([128, 1], f32, name="Bcol", tag="Bcol")
                  nc.vector.tensor_tensor(out=Acol[:, :], in0=z[:, :],
                                          in1=c_bnp[n - 1][:, 0:1], op=ALU.mult)
                  nc.vector.tensor_tensor(out=h[:, :], in0=mom[:, 0:1],
                                          in1=Acol[:, :], op=ALU.mult)
                  nc.vector.tensor_tensor(out=Bcol[:, :],
                                          in0=c_bnp[n - 1][:, 1:2],
                                          in1=h[:, :], op=ALU.subtract)
                  # silu(A*y + B) in one ACT op per tile
                  nxa = xpool.tile([128, 544], f32, name="xa", tag="xa")
                  nxb = xpool.tile([128, 544], f32, name="xb", tag="xb")
                  for xh, ps in ((nxa, pss[0]), (nxb, pss[1])):
                      nc.vector.memset(xh[:, 0:PAD], 0.0)
                      nc.vector.memset(xh[:, PAD + 512:], 0.0)
                      nc.scalar.activation(xh[:, PAD:PAD + 512], ps[:, :],
                                           AF.Silu, bias=Bcol[:, :],
                                           scale=Acol[:, :])
                  xa, xb = nxa, nxb

              if phase == 'layers':
                  return
              # ---- note gather + pooling ----
              for Ti, xh in enumerate((xa, xb)):
                  po = popool.tile([128, ML], f32, name="po", tag="po")
                  for j, (o, L, _) in enumerate(notes):
                      nc.tensor.matmul(po[:, 0:L], c_gid[j][:, :],
                                       xh[:, PAD + o:PAD + o + L],
                                       start=(j == 0), stop=(j == NN - 1))
                  osb = tpool.tile([128, ML], f32, name="osb", tag="osb")
                  nc.scalar.copy(out=osb[:, :], in_=po[:, :])
                  nc.sync.dma_start(
                      out=out_ext[2 * Ti:2 * Ti + 2].rearrange(
                          "s c r -> (s c) r"),
                      in_=osb[:, :])


            for _ in range(loop):
                one_pass()

    nc.compile()
    names = ["wav", "w0lT", "b0c", "l1T", "l2T", "l3T", "l4T", "bnp", "gidT",
             "foldT"]
    return nc, names, ML, NN


# ---------------------------------------------------------------- entry


def kernel(wav, onset_flags, w0, b0,
           dw1, pw1, g1, bt1, dw2, pw2, g2, bt2,
           dw3, pw3, g3, bt3, dw4, pw4, g4, bt4):
    wav = np.asarray(wav, np.float32)
    onset_flags = np.asarray(onset_flags, np.int32)
    w0 = np.asarray(w0, np.float32)
    b0 = np.asarray(b0, np.float32)
    dws = [np.asarray(x, np.float32) for x in (dw1, dw2, dw3, dw4)]
    pws = [np.asarray(x, np.float32) for x in (pw1, pw2, pw3, pw4)]
    gs = [np.asarray(x, np.float32) for x in (g1, g2, g3, g4)]
    bts = [np.asarray(x, np.float32) for x in (bt1, bt2, bt3, bt4)]

    flags = onset_flags[:, 0, :]
    uniform = bool((flags == flags[0:1]).all())
    if wav.shape != (B, 1, T_WAV) or not uniform:
        return _numpy_reference(wav, onset_flags, w0, b0, dws, pws, gs, bts)

    offs, lens, wnt = _note_plan(flags[0])
    if len(offs) == 0:
        return np.zeros((B, 64, P), np.float32)

    key = (tuple(offs), tuple(lens))
    if key not in _CACHE:
        _CACHE[key] = _build((tuple(offs), tuple(lens)))
    nc, names, ML, NN = _CACHE[key]

    consts = _pack_consts(w0, b0, dws, pws, gs, bts, wnt)
    wav2 = wav.reshape(B, T_WAV)
    in_maps = []
    for c in range(N_CORES):
        m = {"wav": np.ascontiguousarray(wav2[B_LOC * c:B_LOC * (c + 1)])}
        for k in names[1:]:
            m[k] = consts[k]
        in_maps.append(m)

    from concourse.bass_utils import run_bass_kernel_spmd
    trace = os.environ.get("KERNEL_TRACE", "0") == "1"
    res = run_bass_kernel_spmd(nc, in_maps, list(range(N_CORES)), trace=trace)
    kernel._last = res

    out = np.zeros((B, 64, P), np.float32)
    for c in range(N_CORES):
        out[B_LOC * c:B_LOC * (c + 1), :, :ML] = res.results[c]["out"]
    return out



# revision 11
# speedup vs baseline: 1.0063x; 1.0063x over previous
"""Trainium2 Bass kernel for nn_ADSREncoderV3 (8-core data-parallel).

Pipeline (per core, 4 of 32 samples):
  1. log-RMS envelope: wav tiles squared (ACT/DVE), window-summed via
     ones-vector matmuls into PSUM (one row per sample), Ln applied
     straight out of PSUM into the conv1 rhs layout -- no DRAM round
     trip for the transpose.
  2. conv1(k=3) + GELU as a single K=12 fp32r matmul.
  3. 4x dsconv blocks: depthwise+pointwise fused into 3 shifted fp32r
     matmuls (PSUM-accumulated), training-mode BatchNorm with exact
     cross-core stats via a [128,2] AllReduce, SiLU via one ACT op with
     the BN affine folded into scale/bias.
  4. note gather + length-weighted pooling as PSUM-accumulated bf16
     matmuls with wnt-scaled identity weights.

All constants ship as one f32 blob + one bf16 blob (two DMAs total).
Output: (32, 64, 512) f32; columns >= max note length are zero.
"""

import os
import numpy as np

HOP = 512
N_MAX = 16
EPS = 1e-7
BN_EPS = 1e-5
B = 32
T_WAV = 262144
P = 512
N_CORES = 8
B_LOC = B // N_CORES  # 4
PAD = 16  # zero pad columns on each side of x tiles (max dilation is 8)

WB = 1664  # bf16 weight blob columns
OFF_W0 = 0
OFF_L1 = 128
OFF_LN = 512   # + ((n-2)*3+k)*128 for n in 2..4
SB = 137   # f32 scalar blob columns
OFF_FOLD = 0
OFF_B0 = 128
OFF_BNP = 129  # + 2*(n-1)

_CACHE = {}
_MAGIC = 0x5F3759DF  # rsqrt seed


# ---------------------------------------------------------------- host plan


def _note_plan(flags_row):
    """Replicates reference note bookkeeping for one onset pattern.

    Returns (offsets, lengths, wnt) for the first N_MAX notes."""
    pos = np.nonzero(flags_row)[0]
    if len(pos) == 0:
        return [], [], []
    pos = pos.tolist()
    ends = pos[1:] + [P]
    offs, lens = [], []
    for n, (o, e) in enumerate(zip(pos, ends)):
        if n >= N_MAX:
            break
        offs.append(int(o))
        lens.append(int(e - o))
    tot = float(sum(lens)) + EPS
    wnt = [l / tot for l in lens]
    return offs, lens, wnt


def _numpy_reference(wav, onset_flags, w0, b0, dws, pws, gs, bts):
    """Exact numpy fallback (used only if inputs deviate from the
    expected uniform-onset-pattern shape)."""
    Bn, _, Tn = wav.shape
    Pn = Tn // HOP
    rms = np.sqrt(np.mean((wav * wav).reshape(Bn, 1, Pn, HOP), axis=-1) + EPS)
    x = np.log(rms + EPS)  # (B,1,P)

    def conv1d(x, w, b=None, dilation=1, groups=1):
        k = w.shape[-1]
        pad = (k - 1) // 2 * dilation
        Bi, Ci, Pi = x.shape
        Co = w.shape[0]
        xp = np.pad(x, ((0, 0), (0, 0), (pad, pad)))
        y = np.zeros((Bi, Co, Pi), np.float32)
        cig = Ci // groups
        cog = Co // groups
        for g in range(groups):
            xs = xp[:, g * cig:(g + 1) * cig]
            wg = w[g * cog:(g + 1) * cog]
            for kk in range(k):
                seg = xs[:, :, kk * dilation: kk * dilation + Pi]
                y[:, g * cog:(g + 1) * cog] += np.einsum(
                    "bip,oi->bop", seg, wg[:, :, kk])
        if b is not None:
            y += b[None, :, None]
        return y

    from math import erf
    verf = np.vectorize(lambda v: erf(v), otypes=[np.float64])
    y = conv1d(x, w0, b0)
    x = (0.5 * y * (1.0 + verf(y / np.sqrt(2.0)))).astype(np.float32)
    for i, (dw, pw, g, bt) in enumerate(zip(dws, pws, gs, bts)):
        c = x.shape[1]
        d = 1 << i
        x = conv1d(x, dw, dilation=d, groups=c)
        x = conv1d(x, pw)
        mu = x.mean(axis=(0, 2), keepdims=True)
        var = x.var(axis=(0, 2), keepdims=True)
        x = (x - mu) / np.sqrt(var + BN_EPS) * g[None, :, None] + bt[None, :, None]
        x = x * (1.0 / (1.0 + np.exp(-x)))
    out = np.zeros((Bn, x.shape[1], Pn), np.float32)
    for b_i in range(Bn):
        offs, lens, wnt = _note_plan(onset_flags[b_i, 0])
        for o, l, w in zip(offs, lens, wnt):
            out[b_i, :, :l] += w * x[b_i, :, o:o + l]
    return out


def _pack_consts(w0, b0, dws, pws, gs, bts, wnt):
    """Build the three device constant blobs."""
    f32 = np.float32
    cblob = np.zeros((128, WB), f32)
    sblob = np.zeros((128, SB), f32)
    # conv1: lhsT (12,128): row q=(t,tau) -> col m=(t,co); 0.5 folds log(sqrt)
    w0h = 0.5 * w0[:, 0, :]  # (32, 3)
    for t in range(4):
        for tau in range(3):
            cblob[3 * t + tau, OFF_W0 + 32 * t:OFF_W0 + 32 * t + 32] = w0h[:, tau]
    sblob[:, OFF_B0] = np.tile(b0.astype(f32), 4)

    # layer 1: 3 x (64, 128): row ci32-block t -> col (t, co64)
    M1 = [pws[0][:, :, 0] * dws[0][None, :, 0, k] for k in range(3)]  # (64,32)
    for k in range(3):
        for t in range(2):
            cblob[32 * t:32 * t + 32,
                  OFF_L1 + 128 * k + 64 * t:OFF_L1 + 128 * k + 64 * t + 64] = \
                M1[k].T
    for n in (1, 2, 3):
        Mk = [pws[n][:, :, 0] * dws[n][None, :, 0, k] for k in range(3)]
        for k in range(3):
            off = OFF_LN + ((n - 1) * 3 + k) * 128
            for t in range(2):
                cblob[64 * t:64 * t + 64, off + 64 * t:off + 64 * t + 64] = \
                    Mk[k].T
    sblob[:, OFF_FOLD:OFF_FOLD + 128] = np.tile(np.eye(64, dtype=f32), (2, 2))
    for n in range(4):
        sblob[:, OFF_BNP + 2 * n] = np.tile(gs[n], 2)
        sblob[:, OFF_BNP + 2 * n + 1] = np.tile(bts[n], 2)

    NN = len(wnt)
    gblob = np.zeros((128, max(NN, 1) * 128), f32)
    for j in range(NN):
        np.fill_diagonal(gblob[:, j * 128:(j + 1) * 128], wnt[j])
    return (cblob.astype(np.float16), sblob,
            gblob.astype(np.float16))


# ---------------------------------------------------------------- device


def _build(plan_key, cc_kind="AR", rsqrt_mode="pow"):
    """Build the SPMD Bass program for a given note plan."""
    import concourse.bacc as bacc
    import concourse.mybir as mybir
    import concourse.tile as tile
    from concourse.bass import ts, ds  # noqa: F401

    notes = sorted(zip(plan_key[0], plan_key[1], range(len(plan_key[0]))),
                   key=lambda x: -x[1])  # by length desc
    ML = notes[0][1]
    NN = len(notes)

    f32 = mybir.dt.float32
    f32r = mybir.dt.float32r
    bf16 = mybir.dt.float16
    i32 = mybir.dt.int32
    AF = mybir.ActivationFunctionType
    ALU = mybir.AluOpType
    AX = mybir.AxisListType

    nc = bacc.Bacc("TRN2", target_bir_lowering=False, debug=False,
                   num_devices=N_CORES, num_swdge_queues=4)

    wav = nc.declare_dram_parameter("wav", [B_LOC, T_WAV], f32, isOutput=False)
    p_cblob = nc.declare_dram_parameter("cblob", [128, WB], bf16,
                                        isOutput=False)
    p_sblob = nc.declare_dram_parameter("sblob", [128, SB], f32,
                                        isOutput=False)
    p_gblob = nc.declare_dram_parameter("gblob", [128, max(NN, 1) * 128],
                                        bf16, isOutput=False)
    out_ext = nc.declare_dram_parameter("out", [B_LOC, 64, ML], f32,
                                        isOutput=True)

    with tile.TileContext(nc) as tc:
        with (
            tc.tile_pool(name="cpool", bufs=1) as cpool,
            tc.tile_pool(name="wpool", bufs=4) as wpool,
            tc.tile_pool(name="spool", bufs=4) as spool,
            tc.tile_pool(name="epool", bufs=1) as epool,
            tc.tile_pool(name="xpool", bufs=2) as xpool,
            tc.tile_pool(name="tpool", bufs=2) as tpool,
            tc.tile_pool(name="pspool", bufs=4, space="PSUM") as pspool,
            tc.tile_pool(name="pepool", bufs=1, space="PSUM") as pepool,
            tc.tile_pool(name="pfpool", bufs=1, space="PSUM") as pfpool,
            tc.tile_pool(name="popool", bufs=2, space="PSUM") as popool,
            tc.tile_pool(name="dpool", bufs=1, space="DRAM") as dpool,
        ):
            # ---- wav DMAs first (sync HWDGE ring, back to back) ----
            wts = []
            for t in range(B_LOC):
                wt = wpool.tile([128, 2048], f32, name=f"wt{t}")
                nc.sync.dma_start(
                    out=wt[:, :],
                    in_=wav[t].rearrange("(p f) -> p f", p=128))
                wts.append(wt)
            # ---- const blobs (scalar HWDGE ring, parallel to wav) ----
            c_all = cpool.tile([128, WB], bf16, name="c_all")
            nc.scalar.dma_start(out=c_all[:, :], in_=p_cblob[:, :])
            c_sc = cpool.tile([128, SB], f32, name="c_sc")
            nc.scalar.dma_start(out=c_sc[:, :], in_=p_sblob[:, :])
            c_gid = cpool.tile([128, max(NN, 1) * 128], bf16, name="c_gid")
            nc.scalar.dma_start(out=c_gid[:, :], in_=p_gblob[:, :])

            c_w0lT = c_all[0:12, OFF_W0:OFF_W0 + 128]
            c_l1 = [c_all[0:64, OFF_L1 + 128 * k:OFF_L1 + 128 * (k + 1)]
                    for k in range(3)]
            c_ln = {(n, k): c_all[:, OFF_LN + ((n - 2) * 3 + k) * 128:
                                  OFF_LN + ((n - 2) * 3 + k + 1) * 128]
                    for n in (2, 3, 4) for k in range(3)}
            c_fold = c_sc[:, OFF_FOLD:OFF_FOLD + 128]
            c_b0c = c_sc[:, OFF_B0:OFF_B0 + 1]
            c_bnp = [c_sc[:, OFF_BNP + 2 * n:OFF_BNP + 2 * n + 2]
                     for n in range(4)]

            dmy = epool.tile([1, 1], f32, name="dmy")
            dmyo = epool.tile([1, 1], f32, name="dmyo")
            nc.vector.memset(dmy[:, :], 1.0)

            # conv1 rhs (built in place, no DRAM round trip)
            rhs1 = epool.tile([12, 544], bf16, name="rhs1")
            nc.vector.memset(rhs1[:, :], 0.0)
            c_eps = epool.tile([128, 1], f32, name="c_eps")
            nc.vector.memset(c_eps[:, :], EPS)

            # ---- envelope: per-partition window sums (sums[p,4t+k] is the
            # sum of squares of window 4p+k of sample t) ----
            sums = epool.tile([128, 16], f32, name="sums")
            for t in range(B_LOC):
                for k in range(4):
                    c = 4 * t + k
                    src = wts[t][:, 512 * k:512 * (k + 1)]
                    if c < 12:
                        sq = spool.tile([128, 512], bf16, name="sq", tag="sq")
                        nc.scalar.activation(sq[:, :], src, AF.Square,
                                             accum_out=sums[:, c:c + 1])
                    else:
                        # last sample on DVE so ACT can preload Ln's table
                        sq = spool.tile([128, 512], f32, name="sqv",
                                        tag="sqv")
                        nc.vector.tensor_tensor(out=sq[:, :], in0=src,
                                                in1=src, op=ALU.mult)
                        nc.vector.reduce_sum(out=sums[:, c:c + 1],
                                             in_=sq[:, :], axis=AX.X)
                    if c == 11:
                        nc.scalar.activation(dmyo[:, :], dmy[:, :], AF.Ln)

            # log_rms = 0.5*ln(sum/512 + eps); the 0.5 is folded into w0.
            lr = epool.tile([128, 16], bf16, name="lr")
            nc.scalar.activation(lr[:, :], sums[:, :],
                                 AF.Ln, bias=c_eps[:, :], scale=1.0 / HOP)
            # preload gelu table while lr round-trips through DRAM
            nc.scalar.activation(dmyo[:, :], dmy[:, :], AF.Gelu)
            lr_dram = dpool.tile([4, 512], bf16, name="lr_dram", space="DRAM")
            nc.sync.dma_start(
                out=lr_dram[:, :].rearrange("t (p k) -> p t k", p=128),
                in_=lr[:, :].rearrange("p (t k) -> p t k", k=4))
            for tau in range(3):
                src_lo = max(0, tau - 1)
                src_hi = min(512, 511 + tau)
                dst_lo = PAD + src_lo - (tau - 1)
                L = src_hi - src_lo
                nc.sync.dma_start(
                    out=rhs1[:, dst_lo:dst_lo + L].rearrange(
                        "(t u) f -> u t f", u=3)[tau],
                    in_=lr_dram[:, src_lo:src_hi])

            # ---- conv1 + gelu ----
            ps1 = pspool.tile([128, 512], f32, name="ps", tag="ps")
            nc.tensor.matmul(ps1[:, :], c_w0lT,
                             rhs1[:, PAD:PAD + 512],
                             start=True, stop=True)
            x1a = xpool.tile([64, 544], bf16, name="x1a", bufs=1)
            x1b = xpool.tile([64, 544], bf16, name="x1b", bufs=1)
            for xh, lohi in ((x1a, (0, 64)), (x1b, (64, 128))):
                nc.vector.memset(xh[:, 0:PAD], 0.0)
                nc.vector.memset(xh[:, PAD + 512:], 0.0)
                nc.scalar.activation(xh[:, PAD:PAD + 512],
                                     ps1[lohi[0]:lohi[1], :], AF.Gelu,
                                     bias=c_b0c[lohi[0]:lohi[1], :], scale=1.0)
            # preload silu table (hidden under layer-1 matmuls)
            nc.scalar.activation(dmyo[:, :], dmy[:, :], AF.Silu)

            # ---- dsconv layers ----
            xa, xb = x1a, x1b
            for n in range(1, 5):
                d = 1 << (n - 1)
                lhs = c_l1 if n == 1 else [c_ln[(n, k)] for k in range(3)]
                stat = tpool.tile([128, 4], f32, name="stat", tag="stat")
                pss = []
                for Ti in range(2):
                    ps = pspool.tile([128, 512], f32, name="ps", tag="ps")
                    pss.append(ps)
                    rr = (xa, xb)[Ti]
                    rhs = [rr[:, PAD + o:PAD + o + 512] for o in (-d, 0, d)]
                    nc.tensor.matmul(ps[:, :], lhs[1], rhs[1],
                                     start=True, stop=False)
                    nc.tensor.matmul(ps[:, :], lhs[0], rhs[0],
                                     start=False, stop=False)
                    nc.tensor.matmul(ps[:, :], lhs[2], rhs[2],
                                     start=False, stop=True)
                    nc.vector.reduce_sum(out=stat[:, Ti:Ti + 1], in_=ps[:, :],
                                         axis=AX.X)
                    sq = spool.tile([128, 512], bf16, name="sq", tag="sq")
                    nc.scalar.activation(sq[:, :], ps[:, :], AF.Square,
                                         accum_out=stat[:, 2 + Ti:3 + Ti])
                # fold [sumA,sumB,sqA,sqB] -> (sum,sq), then partition halves
                part = tpool.tile([128, 2], f32, name="part", tag="part")
                sview = stat[:, :].rearrange("p (a b) -> p a b", a=2)
                nc.vector.tensor_tensor(out=part[:, :], in0=sview[:, :, 0],
                                        in1=sview[:, :, 1], op=ALU.add)
                pf = pfpool.tile([128, 2], f32, name="pf", tag="pf")
                nc.tensor.matmul(pf[:, :], c_fold, part[:, :],
                                 start=True, stop=True)
                part2 = tpool.tile([128, 2], f32, name="part2", tag="part2")
                nc.scalar.copy(out=part2[:, :], in_=pf[:, :])

                glob = tpool.tile([128, 2], f32, name="glob", tag="glob")
                ccin = dpool.tile([128, 2], f32, name=f"ccin{n}", space="DRAM")
                nc.sync.dma_start(out=ccin[:, :], in_=part2[:, :])
                if cc_kind == "AR":
                    ccout = dpool.tile([128, 2], f32, name=f"ccout{n}",
                                       space="DRAM", addr_space="Shared")
                    nc.gpsimd.collective_compute(
                        "AllReduce", ALU.add,
                        replica_groups=[list(range(N_CORES))],
                        ins=[ccin[:, :].opt()], outs=[ccout[:, :].opt()])
                    nc.sync.dma_start(out=glob[:, :], in_=ccout[:, :])
                else:
                    ccout = dpool.tile([1024, 2], f32, name=f"ccout{n}",
                                       space="DRAM", addr_space="Shared")
                    nc.gpsimd.collective_compute(
                        "AllGather", ALU.bypass,
                        replica_groups=[list(range(N_CORES))],
                        ins=[ccin[:, :].opt()], outs=[ccout[:, :].opt()])
                    gath = tpool.tile([128, 16], f32, name="gath", tag="gath")
                    nc.sync.dma_start(
                        out=gath[:, :].rearrange("c (r s) -> c r s", s=2),
                        in_=ccout[:, :].rearrange("(r c) s -> c r s", c=128))
                    nc.vector.reduce_sum(
                        out=glob[:, :],
                        in_=gath[:, :].rearrange("c (r s) -> c s r", s=2),
                        axis=AX.X)

                # BN affine: A = g*rsqrt(var+eps), Bb = bt - mu*A
                mom = tpool.tile([128, 2], f32, name="mom", tag="mom")
                nc.vector.tensor_scalar_mul(mom[:, :], glob[:, :],
                                            1.0 / (B * P))
                var = tpool.tile([128, 1], f32, name="var", tag="var")
                nc.vector.tensor_tensor(out=var[:, :], in0=mom[:, 0:1],
                                        in1=mom[:, 0:1], op=ALU.mult)
                nc.vector.tensor_tensor(out=var[:, :], in0=mom[:, 1:2],
                                        in1=var[:, :], op=ALU.subtract)
                z = tpool.tile([128, 1], f32, name="z", tag="z")
                if rsqrt_mode == "pow":
                    nc.vector.tensor_scalar(
                        out=z[:, :], in0=var[:, :],
                        scalar1=BN_EPS, scalar2=-0.5,
                        op0=ALU.add, op1=ALU.pow)
                else:
                    veps = tpool.tile([128, 1], f32, name="veps", tag="veps")
                    nc.vector.tensor_scalar_add(veps[:, :], var[:, :], BN_EPS)
                    h0 = tpool.tile([128, 1], f32, name="h0", tag="h0")
                    nc.vector.tensor_scalar(
                        out=h0[:, :].bitcast(i32),
                        in0=veps[:, :].bitcast(i32),
                        scalar1=1, scalar2=None,
                        op0=ALU.logical_shift_right)
                    nc.vector.tensor_scalar(
                        out=z[:, :].bitcast(i32),
                        in0=h0[:, :].bitcast(i32),
                        scalar1=-1, scalar2=_MAGIC,
                        op0=ALU.mult, op1=ALU.add)
                    for _ in range(3):
                        nc.vector.tensor_tensor(out=h0[:, :], in0=z[:, :],
                                                in1=z[:, :], op=ALU.mult)
                        nc.vector.tensor_tensor(out=h0[:, :], in0=h0[:, :],
                                                in1=veps[:, :], op=ALU.mult)
                        nc.vector.tensor_scalar(out=h0[:, :], in0=h0[:, :],
                                                scalar1=-0.5, scalar2=1.5,
                                                op0=ALU.mult, op1=ALU.add)
                        nc.vector.tensor_tensor(out=z[:, :], in0=z[:, :],
                                                in1=h0[:, :], op=ALU.mult)
                Acol = tpool.tile([128, 1], f32, name="Acol", tag="Acol")
                Bcol = tpool.tile([128, 1], f32, name="Bcol", tag="Bcol")
                h = tpool.tile([128, 1], f32, name="h", tag="h")
                nc.vector.tensor_tensor(out=Acol[:, :], in0=z[:, :],
                                        in1=c_bnp[n - 1][:, 0:1], op=ALU.mult)
                nc.vector.tensor_tensor(out=h[:, :], in0=mom[:, 0:1],
                                        in1=Acol[:, :], op=ALU.mult)
                nc.vector.tensor_tensor(out=Bcol[:, :],
                                        in0=c_bnp[n - 1][:, 1:2],
                                        in1=h[:, :], op=ALU.subtract)
                # silu(A*y + B) in one ACT op per tile
                nxa = xpool.tile([128, 544], bf16, name="xa", tag="xa")
                nxb = xpool.tile([128, 544], bf16, name="xb", tag="xb")
                for xh, ps in ((nxa, pss[0]), (nxb, pss[1])):
                    nc.vector.memset(xh[:, 0:PAD], 0.0)
                    nc.vector.memset(xh[:, PAD + 512:], 0.0)
                    nc.scalar.activation(xh[:, PAD:PAD + 512], ps[:, :],
                                         AF.Silu, bias=Bcol[:, :],
                                         scale=Acol[:, :])
                xa, xb = nxa, nxb

            # ---- note gather + pooling (bf16 matmuls) ----
            for Ti, xh in enumerate((xa, xb)):
                po = popool.tile([128, ML], f32, name="po", tag="po")
                for j, (o, L, jid) in enumerate(notes):
                    nc.tensor.matmul(po[:, 0:L],
                                     c_gid[:, jid * 128:(jid + 1) * 128],
                                     xh[:, PAD + o:PAD + o + L],
                                     start=(j == 0), stop=(j == NN - 1))
                osb = tpool.tile([128, ML], f32, name="osb", tag="osb")
                nc.scalar.copy(out=osb[:, :], in_=po[:, :])
                nc.sync.dma_start(
                    out=out_ext[2 * Ti:2 * Ti + 2].rearrange(
                        "s c r -> (s c) r"),
                    in_=osb[:, :])

    nc.compile()
    return nc, ML, NN


# ---------------------------------------------------------------- entry


def kernel(wav, onset_flags, w0, b0,
           dw1, pw1, g1, bt1, dw2, pw2, g2, bt2,
           dw3, pw3, g3, bt3, dw4, pw4, g4, bt4):
    wav = np.asarray(wav, np.float32)
    onset_flags = np.asarray(onset_flags, np.int32)
    w0 = np.asarray(w0, np.float32)
    b0 = np.asarray(b0, np.float32)
    dws = [np.asarray(x, np.float32) for x in (dw1, dw2, dw3, dw4)]
    pws = [np.asarray(x, np.float32) for x in (pw1, pw2, pw3, pw4)]
    gs = [np.asarray(x, np.float32) for x in (g1, g2, g3, g4)]
    bts = [np.asarray(x, np.float32) for x in (bt1, bt2, bt3, bt4)]

    flags = onset_flags[:, 0, :]
    uniform = bool((flags == flags[0:1]).all())
    if wav.shape != (B, 1, T_WAV) or not uniform:
        return _numpy_reference(wav, onset_flags, w0, b0, dws, pws, gs, bts)

    offs, lens, wnt = _note_plan(flags[0])
    if len(offs) == 0:
        return np.zeros((B, 64, P), np.float32)

    cc_kind = os.environ.get("KERNEL_CC_KIND", "AR")
    rsqrt_mode = os.environ.get("KERNEL_RSQRT", "newton")
    key = (tuple(offs), tuple(lens), cc_kind, rsqrt_mode)
    if key not in _CACHE:
        _CACHE[key] = _build((tuple(offs), tuple(lens)), cc_kind, rsqrt_mode)
    nc, ML, NN = _CACHE[key]

    cblob, sblob, gblob = _pack_consts(w0, b0, dws, pws, gs, bts, wnt)
    wav2 = wav.reshape(B, T_WAV)
    in_maps = []
    for c in range(N_CORES):
        in_maps.append({
            "wav": np.ascontiguousarray(wav2[B_LOC * c:B_LOC * (c + 1)]),
            "cblob": cblob,
            "sblob": sblob,
            "gblob": gblob,
        })

    from concourse.bass_utils import run_bass_kernel_spmd
    trace = os.environ.get("KERNEL_TRACE", "0") == "1"
    res = run_bass_kernel_spmd(nc, in_maps, list(range(N_CORES)), trace=trace)
    kernel._last = res

    out = np.zeros((B, 64, P), np.float32)
    for c in range(N_CORES):
        out[B_LOC * c:B_LOC * (c + 1), :, :ML] = res.results[c]["out"]
    return out


# revision 12
# speedup vs baseline: 1.5055x; 1.4961x over previous
"""Trainium2 Bass kernel for nn_ADSREncoderV3 (8-core data-parallel).

Pipeline (per core, 4 of 32 samples):
  1. log-RMS envelope: per-partition window sums of squares (ACT accum
     for samples 0-2, DVE for sample 3 so ACT can preload Ln's table),
     Ln, then a DRAM round trip to transpose into the conv1 rhs layout.
  2. conv1(k=3) + GELU as a single K=12 fp16 matmul.
  3. 4x dsconv blocks: depthwise+pointwise fused into 3 shifted fp16
     matmuls (PSUM-accumulated), training-mode BatchNorm with exact
     cross-core stats via a [128,2] AllReduce (fp32 partial sums),
     SiLU via one ACT op with the BN affine folded into scale/bias.
  4. note gather + length-weighted pooling as PSUM-accumulated fp16
     matmuls with wnt-scaled identity weights.

Constants ship as fp16 weight + f32 scalar + fp16 gather blobs (three
DMAs total).
Output: (32, 64, 512) f32; columns >= max note length are zero.
"""

import os
import numpy as np

HOP = 512
N_MAX = 16
EPS = 1e-7
BN_EPS = 1e-5
B = 32
T_WAV = 262144
P = 512
N_CORES = 8
B_LOC = B // N_CORES  # 4
PAD = 16  # zero pad columns on each side of x tiles (max dilation is 8)

WB = 1664  # bf16 weight blob columns
OFF_W0 = 0
OFF_L1 = 128
OFF_LN = 512   # + ((n-2)*3+k)*128 for n in 2..4
SB = 137   # f32 scalar blob columns
OFF_FOLD = 0
OFF_B0 = 128
OFF_BNP = 129  # + 2*(n-1)

_CACHE = {}
_MAGIC = 0x5F3759DF  # rsqrt seed


# ---------------------------------------------------------------- host plan


def _note_plan(flags_row):
    """Replicates reference note bookkeeping for one onset pattern.

    Returns (offsets, lengths, wnt) for the first N_MAX notes."""
    pos = np.nonzero(flags_row)[0]
    if len(pos) == 0:
        return [], [], []
    pos = pos.tolist()
    ends = pos[1:] + [P]
    offs, lens = [], []
    for n, (o, e) in enumerate(zip(pos, ends)):
        if n >= N_MAX:
            break
        offs.append(int(o))
        lens.append(int(e - o))
    tot = float(sum(lens)) + EPS
    wnt = [l / tot for l in lens]
    return offs, lens, wnt


def _numpy_reference(wav, onset_flags, w0, b0, dws, pws, gs, bts):
    """Exact numpy fallback (used only if inputs deviate from the
    expected uniform-onset-pattern shape)."""
    Bn, _, Tn = wav.shape
    Pn = Tn // HOP
    rms = np.sqrt(np.mean((wav * wav).reshape(Bn, 1, Pn, HOP), axis=-1) + EPS)
    x = np.log(rms + EPS)  # (B,1,P)

    def conv1d(x, w, b=None, dilation=1, groups=1):
        k = w.shape[-1]
        pad = (k - 1) // 2 * dilation
        Bi, Ci, Pi = x.shape
        Co = w.shape[0]
        xp = np.pad(x, ((0, 0), (0, 0), (pad, pad)))
        y = np.zeros((Bi, Co, Pi), np.float32)
        cig = Ci // groups
        cog = Co // groups
        for g in range(groups):
            xs = xp[:, g * cig:(g + 1) * cig]
            wg = w[g * cog:(g + 1) * cog]
            for kk in range(k):
                seg = xs[:, :, kk * dilation: kk * dilation + Pi]
                y[:, g * cog:(g + 1) * cog] += np.einsum(
                    "bip,oi->bop", seg, wg[:, :, kk])
        if b is not None:
            y += b[None, :, None]
        return y

    from math import erf
    verf = np.vectorize(lambda v: erf(v), otypes=[np.float64])
    y = conv1d(x, w0, b0)
    x = (0.5 * y * (1.0 + verf(y / np.sqrt(2.0)))).astype(np.float32)
    for i, (dw, pw, g, bt) in enumerate(zip(dws, pws, gs, bts)):
        c = x.shape[1]
        d = 1 << i
        x = conv1d(x, dw, dilation=d, groups=c)
        x = conv1d(x, pw)
        mu = x.mean(axis=(0, 2), keepdims=True)
        var = x.var(axis=(0, 2), keepdims=True)
        x = (x - mu) / np.sqrt(var + BN_EPS) * g[None, :, None] + bt[None, :, None]
        x = x * (1.0 / (1.0 + np.exp(-x)))
    out = np.zeros((Bn, x.shape[1], Pn), np.float32)
    for b_i in range(Bn):
        offs, lens, wnt = _note_plan(onset_flags[b_i, 0])
        for o, l, w in zip(offs, lens, wnt):
            out[b_i, :, :l] += w * x[b_i, :, o:o + l]
    return out


def _pack_consts(w0, b0, dws, pws, gs, bts, wnt):
    """Build the three device constant blobs."""
    f32 = np.float32
    cblob = np.zeros((128, WB), f32)
    sblob = np.zeros((128, SB), f32)
    # conv1: lhsT (12,128): row q=(t,tau) -> col m=(t,co); 0.5 folds log(sqrt)
    w0h = 0.5 * w0[:, 0, :]  # (32, 3)
    for t in range(4):
        for tau in range(3):
            cblob[3 * t + tau, OFF_W0 + 32 * t:OFF_W0 + 32 * t + 32] = w0h[:, tau]
    sblob[:, OFF_B0] = np.tile(b0.astype(f32), 4)

    # layer 1: 3 x (64, 128): row ci32-block t -> col (t, co64)
    M1 = [pws[0][:, :, 0] * dws[0][None, :, 0, k] for k in range(3)]  # (64,32)
    for k in range(3):
        for t in range(2):
            cblob[32 * t:32 * t + 32,
                  OFF_L1 + 128 * k + 64 * t:OFF_L1 + 128 * k + 64 * t + 64] = \
                M1[k].T
    for n in (1, 2, 3):
        Mk = [pws[n][:, :, 0] * dws[n][None, :, 0, k] for k in range(3)]
        for k in range(3):
            off = OFF_LN + ((n - 1) * 3 + k) * 128
            for t in range(2):
                cblob[64 * t:64 * t + 64, off + 64 * t:off + 64 * t + 64] = \
                    Mk[k].T
    sblob[:, OFF_FOLD:OFF_FOLD + 128] = np.tile(np.eye(64, dtype=f32), (2, 2))
    for n in range(4):
        sblob[:, OFF_BNP + 2 * n] = np.tile(gs[n], 2)
        sblob[:, OFF_BNP + 2 * n + 1] = np.tile(bts[n], 2)

    NN = len(wnt)
    gblob = np.zeros((128, max(NN, 1) * 128), f32)
    for j in range(NN):
        np.fill_diagonal(gblob[:, j * 128:(j + 1) * 128], wnt[j])
    return (cblob.astype(np.float16), sblob,
            gblob.astype(np.float16))


# ---------------------------------------------------------------- device


def _build(plan_key, cc_kind="AR", rsqrt_mode="pow"):
    """Build the SPMD Bass program for a given note plan."""
    import concourse.bacc as bacc
    import concourse.mybir as mybir
    import concourse.tile as tile
    from concourse.bass import ts, ds  # noqa: F401

    notes = sorted(zip(plan_key[0], plan_key[1], range(len(plan_key[0]))),
                   key=lambda x: -x[1])  # by length desc
    ML = notes[0][1]
    NN = len(notes)

    f32 = mybir.dt.float32
    f32r = mybir.dt.float32r
    bf16 = mybir.dt.float16
    i32 = mybir.dt.int32
    AF = mybir.ActivationFunctionType
    ALU = mybir.AluOpType
    AX = mybir.AxisListType

    nc = bacc.Bacc("TRN2", target_bir_lowering=False, debug=False,
                   num_devices=N_CORES, num_swdge_queues=4)

    wav = nc.declare_dram_parameter("wav", [B_LOC, T_WAV], f32, isOutput=False)
    p_cblob = nc.declare_dram_parameter("cblob", [128, WB], bf16,
                                        isOutput=False)
    p_sblob = nc.declare_dram_parameter("sblob", [128, SB], f32,
                                        isOutput=False)
    p_gblob = nc.declare_dram_parameter("gblob", [128, max(NN, 1) * 128],
                                        bf16, isOutput=False)
    out_ext = nc.declare_dram_parameter("out", [B_LOC, 64, ML], f32,
                                        isOutput=True)

    with tile.TileContext(nc) as tc:
        with (
            tc.tile_pool(name="cpool", bufs=1) as cpool,
            tc.tile_pool(name="wpool", bufs=4) as wpool,
            tc.tile_pool(name="spool", bufs=4) as spool,
            tc.tile_pool(name="epool", bufs=1) as epool,
            tc.tile_pool(name="xpool", bufs=2) as xpool,
            tc.tile_pool(name="tpool", bufs=2) as tpool,
            tc.tile_pool(name="pspool", bufs=4, space="PSUM") as pspool,
            tc.tile_pool(name="pepool", bufs=1, space="PSUM") as pepool,
            tc.tile_pool(name="pfpool", bufs=1, space="PSUM") as pfpool,
            tc.tile_pool(name="popool", bufs=2, space="PSUM") as popool,
            tc.tile_pool(name="dpool", bufs=1, space="DRAM") as dpool,
        ):
            # ---- wav DMAs first (sync HWDGE ring, back to back) ----
            wts = []
            for t in range(B_LOC):
                wt = wpool.tile([128, 2048], f32, name=f"wt{t}")
                nc.sync.dma_start(
                    out=wt[:, :],
                    in_=wav[t].rearrange("(p f) -> p f", p=128))
                wts.append(wt)
            # ---- const blobs (scalar HWDGE ring, parallel to wav) ----
            c_all = cpool.tile([128, WB], bf16, name="c_all")
            nc.scalar.dma_start(out=c_all[:, :], in_=p_cblob[:, :])
            c_sc = cpool.tile([128, SB], f32, name="c_sc")
            nc.scalar.dma_start(out=c_sc[:, :], in_=p_sblob[:, :])
            c_gid = cpool.tile([128, max(NN, 1) * 128], bf16, name="c_gid")
            nc.scalar.dma_start(out=c_gid[:, :], in_=p_gblob[:, :])

            c_w0lT = c_all[0:12, OFF_W0:OFF_W0 + 128]
            c_l1 = [c_all[0:64, OFF_L1 + 128 * k:OFF_L1 + 128 * (k + 1)]
                    for k in range(3)]
            c_ln = {(n, k): c_all[:, OFF_LN + ((n - 2) * 3 + k) * 128:
                                  OFF_LN + ((n - 2) * 3 + k + 1) * 128]
                    for n in (2, 3, 4) for k in range(3)}
            c_fold = c_sc[:, OFF_FOLD:OFF_FOLD + 128]
            c_b0c = c_sc[:, OFF_B0:OFF_B0 + 1]
            c_bnp = [c_sc[:, OFF_BNP + 2 * n:OFF_BNP + 2 * n + 2]
                     for n in range(4)]

            dmy = epool.tile([1, 1], f32, name="dmy")
            dmyo = epool.tile([1, 1], f32, name="dmyo")
            nc.vector.memset(dmy[:, :], 1.0)

            # conv1 rhs (built in place, no DRAM round trip)
            rhs1 = epool.tile([12, 544], bf16, name="rhs1")
            nc.vector.memset(rhs1[:, :], 0.0)
            c_eps = epool.tile([128, 1], f32, name="c_eps")
            nc.vector.memset(c_eps[:, :], EPS)

            # ---- envelope: per-partition window sums (sums[p,4t+k] is the
            # sum of squares of window 4p+k of sample t) ----
            sums = epool.tile([128, 16], f32, name="sums")
            for t in range(B_LOC):
                for k in range(4):
                    c = 4 * t + k
                    src = wts[t][:, 512 * k:512 * (k + 1)]
                    if c < 12:
                        sq = spool.tile([128, 512], bf16, name="sq", tag="sq")
                        nc.scalar.activation(sq[:, :], src, AF.Square,
                                             accum_out=sums[:, c:c + 1])
                    else:
                        # last sample on DVE so ACT can preload Ln's table
                        sq = spool.tile([128, 512], f32, name="sqv",
                                        tag="sqv")
                        nc.vector.tensor_tensor(out=sq[:, :], in0=src,
                                                in1=src, op=ALU.mult)
                        nc.vector.reduce_sum(out=sums[:, c:c + 1],
                                             in_=sq[:, :], axis=AX.X)
                    if c == 11:
                        nc.scalar.activation(dmyo[:, :], dmy[:, :], AF.Ln)

            # log_rms = 0.5*ln(sum/512 + eps); the 0.5 is folded into w0.
            lr = epool.tile([128, 16], bf16, name="lr")
            nc.scalar.activation(lr[:, :], sums[:, :],
                                 AF.Ln, bias=c_eps[:, :], scale=1.0 / HOP)
            # preload gelu table while lr round-trips through DRAM
            nc.scalar.activation(dmyo[:, :], dmy[:, :], AF.Gelu)
            lr_dram = dpool.tile([4, 512], bf16, name="lr_dram", space="DRAM")
            nc.sync.dma_start(
                out=lr_dram[:, :].rearrange("t (p k) -> p t k", p=128),
                in_=lr[:, :].rearrange("p (t k) -> p t k", k=4))
            for tau in range(3):
                src_lo = max(0, tau - 1)
                src_hi = min(512, 511 + tau)
                dst_lo = PAD + src_lo - (tau - 1)
                L = src_hi - src_lo
                nc.sync.dma_start(
                    out=rhs1[:, dst_lo:dst_lo + L].rearrange(
                        "(t u) f -> u t f", u=3)[tau],
                    in_=lr_dram[:, src_lo:src_hi])

            # ---- conv1 + gelu ----
            ps1 = pspool.tile([128, 512], f32, name="ps", tag="ps")
            nc.tensor.matmul(ps1[:, :], c_w0lT,
                             rhs1[:, PAD:PAD + 512],
                             start=True, stop=True)
            x1a = xpool.tile([64, 544], bf16, name="x1a", bufs=1)
            x1b = xpool.tile([64, 544], bf16, name="x1b", bufs=1)
            for xh, lohi in ((x1a, (0, 64)), (x1b, (64, 128))):
                nc.vector.memset(xh[:, 0:PAD], 0.0)
                nc.vector.memset(xh[:, PAD + 512:], 0.0)
                nc.scalar.activation(xh[:, PAD:PAD + 512],
                                     ps1[lohi[0]:lohi[1], :], AF.Gelu,
                                     bias=c_b0c[lohi[0]:lohi[1], :], scale=1.0)
            # preload silu table (hidden under layer-1 matmuls)
            nc.scalar.activation(dmyo[:, :], dmy[:, :], AF.Silu)

            # ---- dsconv layers ----
            xa, xb = x1a, x1b
            for n in range(1, 5):
                d = 1 << (n - 1)
                lhs = c_l1 if n == 1 else [c_ln[(n, k)] for k in range(3)]
                stat = tpool.tile([128, 4], f32, name="stat", tag="stat")
                pss = []
                for Ti in range(2):
                    ps = pspool.tile([128, 512], f32, name="ps", tag="ps")
                    pss.append(ps)
                    rr = (xa, xb)[Ti]
                    rhs = [rr[:, PAD + o:PAD + o + 512] for o in (-d, 0, d)]
                    nc.tensor.matmul(ps[:, :], lhs[1], rhs[1],
                                     start=True, stop=False)
                    nc.tensor.matmul(ps[:, :], lhs[0], rhs[0],
                                     start=False, stop=False)
                    nc.tensor.matmul(ps[:, :], lhs[2], rhs[2],
                                     start=False, stop=True)
                    nc.vector.reduce_sum(out=stat[:, Ti:Ti + 1], in_=ps[:, :],
                                         axis=AX.X)
                    sq = spool.tile([128, 512], bf16, name="sq", tag="sq")
                    nc.scalar.activation(sq[:, :], ps[:, :], AF.Square,
                                         accum_out=stat[:, 2 + Ti:3 + Ti])
                # fold [sumA,sumB,sqA,sqB] -> (sum,sq), then partition halves
                part = tpool.tile([128, 2], f32, name="part", tag="part")
                sview = stat[:, :].rearrange("p (a b) -> p a b", a=2)
                nc.vector.tensor_tensor(out=part[:, :], in0=sview[:, :, 0],
                                        in1=sview[:, :, 1], op=ALU.add)
                pf = pfpool.tile([128, 2], f32, name="pf", tag="pf")
                nc.tensor.matmul(pf[:, :], c_fold, part[:, :],
                                 start=True, stop=True)
                part2 = tpool.tile([128, 2], f32, name="part2", tag="part2")
                nc.scalar.copy(out=part2[:, :], in_=pf[:, :])

                glob = tpool.tile([128, 2], f32, name="glob", tag="glob")
                ccin = dpool.tile([128, 2], f32, name=f"ccin{n}", space="DRAM")
                nc.sync.dma_start(out=ccin[:, :], in_=part2[:, :])
                if cc_kind == "AR":
                    ccout = dpool.tile([128, 2], f32, name=f"ccout{n}",
                                       space="DRAM", addr_space="Shared")
                    nc.gpsimd.collective_compute(
                        "AllReduce", ALU.add,
                        replica_groups=[list(range(N_CORES))],
                        ins=[ccin[:, :].opt()], outs=[ccout[:, :].opt()])
                    nc.sync.dma_start(out=glob[:, :], in_=ccout[:, :])
                else:
                    ccout = dpool.tile([1024, 2], f32, name=f"ccout{n}",
                                       space="DRAM", addr_space="Shared")
                    nc.gpsimd.collective_compute(
                        "AllGather", ALU.bypass,
                        replica_groups=[list(range(N_CORES))],
                        ins=[ccin[:, :].opt()], outs=[ccout[:, :].opt()])
                    gath = tpool.tile([128, 16], f32, name="gath", tag="gath")
                    nc.sync.dma_start(
                        out=gath[:, :].rearrange("c (r s) -> c r s", s=2),
                        in_=ccout[:, :].rearrange("(r c) s -> c r s", c=128))
                    nc.vector.reduce_sum(
                        out=glob[:, :],
                        in_=gath[:, :].rearrange("c (r s) -> c s r", s=2),
                        axis=AX.X)

                # BN affine: A = g*rsqrt(var+eps), Bb = bt - mu*A
                mom = tpool.tile([128, 2], f32, name="mom", tag="mom")
                nc.vector.tensor_scalar_mul(mom[:, :], glob[:, :],
                                            1.0 / (B * P))
                var = tpool.tile([128, 1], f32, name="var", tag="var")
                nc.vector.tensor_tensor(out=var[:, :], in0=mom[:, 0:1],
                                        in1=mom[:, 0:1], op=ALU.mult)
                nc.vector.tensor_tensor(out=var[:, :], in0=mom[:, 1:2],
                                        in1=var[:, :], op=ALU.subtract)
                z = tpool.tile([128, 1], f32, name="z", tag="z")
                if rsqrt_mode == "pow":
                    nc.vector.tensor_scalar(
                        out=z[:, :], in0=var[:, :],
                        scalar1=BN_EPS, scalar2=-0.5,
                        op0=ALU.add, op1=ALU.pow)
                else:
                    veps = tpool.tile([128, 1], f32, name="veps", tag="veps")
                    nc.vector.tensor_scalar_add(veps[:, :], var[:, :], BN_EPS)
                    h0 = tpool.tile([128, 1], f32, name="h0", tag="h0")
                    nc.vector.tensor_scalar(
                        out=h0[:, :].bitcast(i32),
                        in0=veps[:, :].bitcast(i32),
                        scalar1=1, scalar2=None,
                        op0=ALU.logical_shift_right)
                    nc.vector.tensor_scalar(
                        out=z[:, :].bitcast(i32),
                        in0=h0[:, :].bitcast(i32),
                        scalar1=-1, scalar2=_MAGIC,
                        op0=ALU.mult, op1=ALU.add)
                    for _ in range(2):
                        nc.vector.tensor_tensor(out=h0[:, :], in0=z[:, :],
                                                in1=z[:, :], op=ALU.mult)
                        nc.vector.tensor_tensor(out=h0[:, :], in0=h0[:, :],
                                                in1=veps[:, :], op=ALU.mult)
                        nc.vector.tensor_scalar(out=h0[:, :], in0=h0[:, :],
                                                scalar1=-0.5, scalar2=1.5,
                                                op0=ALU.mult, op1=ALU.add)
                        nc.vector.tensor_tensor(out=z[:, :], in0=z[:, :],
                                                in1=h0[:, :], op=ALU.mult)
                Acol = tpool.tile([128, 1], f32, name="Acol", tag="Acol")
                Bcol = tpool.tile([128, 1], f32, name="Bcol", tag="Bcol")
                h = tpool.tile([128, 1], f32, name="h", tag="h")
                nc.vector.tensor_tensor(out=Acol[:, :], in0=z[:, :],
                                        in1=c_bnp[n - 1][:, 0:1], op=ALU.mult)
                nc.vector.tensor_tensor(out=h[:, :], in0=mom[:, 0:1],
                                        in1=Acol[:, :], op=ALU.mult)
                nc.vector.tensor_tensor(out=Bcol[:, :],
                                        in0=c_bnp[n - 1][:, 1:2],
                                        in1=h[:, :], op=ALU.subtract)
                # silu(A*y + B) in one ACT op per tile
                nxa = xpool.tile([128, 544], bf16, name="xa", tag="xa")
                nxb = xpool.tile([128, 544], bf16, name="xb", tag="xb")
                for xh, ps in ((nxa, pss[0]), (nxb, pss[1])):
                    nc.vector.memset(xh[:, 0:PAD], 0.0)
                    nc.vector.memset(xh[:, PAD + 512:], 0.0)
                    nc.scalar.activation(xh[:, PAD:PAD + 512], ps[:, :],
                                         AF.Silu, bias=Bcol[:, :],
                                         scale=Acol[:, :])
                xa, xb = nxa, nxb

            # ---- note gather + pooling (bf16 matmuls) ----
            for Ti, xh in enumerate((xa, xb)):
                po = popool.tile([128, ML], f32, name="po", tag="po")
                for j, (o, L, jid) in enumerate(notes):
                    nc.tensor.matmul(po[:, 0:L],
                                     c_gid[:, jid * 128:(jid + 1) * 128],
                                     xh[:, PAD + o:PAD + o + L],
                                     start=(j == 0), stop=(j == NN - 1))
                osb = tpool.tile([128, ML], f32, name="osb", tag="osb")
                nc.scalar.copy(out=osb[:, :], in_=po[:, :])
                nc.sync.dma_start(
                    out=out_ext[2 * Ti:2 * Ti + 2].rearrange(
                        "s c r -> (s c) r"),
                    in_=osb[:, :])

    nc.compile()
    return nc, ML, NN


# ---------------------------------------------------------------- entry


def kernel(wav, onset_flags, w0, b0,
           dw1, pw1, g1, bt1, dw2, pw2, g2, bt2,
           dw3, pw3, g3, bt3, dw4, pw4, g4, bt4):
    wav = np.asarray(wav, np.float32)
    onset_flags = np.asarray(onset_flags, np.int32)
    w0 = np.asarray(w0, np.float32)
    b0 = np.asarray(b0, np.float32)
    dws = [np.asarray(x, np.float32) for x in (dw1, dw2, dw3, dw4)]
    pws = [np.asarray(x, np.float32) for x in (pw1, pw2, pw3, pw4)]
    gs = [np.asarray(x, np.float32) for x in (g1, g2, g3, g4)]
    bts = [np.asarray(x, np.float32) for x in (bt1, bt2, bt3, bt4)]

    flags = onset_flags[:, 0, :]
    uniform = bool((flags == flags[0:1]).all())
    if wav.shape != (B, 1, T_WAV) or not uniform:
        return _numpy_reference(wav, onset_flags, w0, b0, dws, pws, gs, bts)

    offs, lens, wnt = _note_plan(flags[0])
    if len(offs) == 0:
        return np.zeros((B, 64, P), np.float32)

    cc_kind = os.environ.get("KERNEL_CC_KIND", "AR")
    rsqrt_mode = os.environ.get("KERNEL_RSQRT", "newton")
    key = (tuple(offs), tuple(lens), cc_kind, rsqrt_mode)
    if key not in _CACHE:
        _CACHE[key] = _build((tuple(offs), tuple(lens)), cc_kind, rsqrt_mode)
    nc, ML, NN = _CACHE[key]

    cblob, sblob, gblob = _pack_consts(w0, b0, dws, pws, gs, bts, wnt)
    wav2 = wav.reshape(B, T_WAV)
    in_maps = []
    for c in range(N_CORES):
        in_maps.append({
            "wav": np.ascontiguousarray(wav2[B_LOC * c:B_LOC * (c + 1)]),
            "cblob": cblob,
            "sblob": sblob,
            "gblob": gblob,
        })

    from concourse.bass_utils import run_bass_kernel_spmd
    trace = os.environ.get("KERNEL_TRACE", "0") == "1"
    res = run_bass_kernel_spmd(nc, in_maps, list(range(N_CORES)), trace=trace)
    kernel._last = res

    out = np.zeros((B, 64, P), np.float32)
    for c in range(N_CORES):
        out[B_LOC * c:B_LOC * (c + 1), :, :ML] = res.results[c]["out"]
    return out
